# revision 4
# baseline (speedup 1.0000x reference)
"""Trainium2 Bass kernel for the DissipativeRINN problem.

Strategy (pure data parallel over batch, 8 cores x 256 batch each):
  - Transposed layout on-chip: activations are [feature, batch] so every
    reference matmul  z = a @ M_T  becomes  psum = M_T.T @ aT  with the
    given *_T matrices used directly as PE stationary weights (lhsT).
  - The RK4 stage states x2/x3/x4 are never materialized: every stage's
    fixed-point offset C_s, the controller output u, and the next-step
    state delta are host-folded into small matrices over the basis
    {[x;y], w1, w2, w3, w4}, so each solve iteration is just matmuls
    into PSUM followed by one ScalarE tanh back to SBUF (bf16).
  - The cold 30-iteration solve is truncated (it converges to fp32 noise
    by ~14 iters) and warm-started from the previous step's w1.
  - Batch is further split into 2 lanes of 128 columns per core, so the
    PE->ACT->PE dependency chain of one lane overlaps the other lane.
  - The value MLP is emitted last and gap-fills idle engine slots.
"""

import numpy as np
import ml_dtypes

bf16 = ml_dtypes.bfloat16

DT = 0.01
B, T, IN, ST, NL, OUT, H = 2048, 32, 16, 16, 128, 8, 64
NCORES = 8
BL = B // NCORES  # 256

# tunables
LANES = int(__import__("os").environ.get("K_LANES", "2"))
N1_COLD = int(__import__("os").environ.get("K_N1_COLD", "15"))
N1_WARM = int(__import__("os").environ.get("K_N1_WARM", "10"))
REUSE = 5
T_STEPS = int(__import__("os").environ.get("K_T", str(T)))
LANE_W = BL // LANES
VCHUNK = 512
NVC = T * BL // VCHUNK  # 16 value chunks


# ---------------------------------------------------------------------------
# host-side math: fold the RK4 stage structure into matrices over the basis
# {XY (32 rows: x rows 0:16, y rows 16:32), W1, W2, W3, W4}
# row-vector convention: quantity = sum_b basis_b @ M[b]
# ---------------------------------------------------------------------------

def _dadd(*ds):
    out = {}
    for d in ds:
        for k, v in d.items():
            out[k] = out.get(k, 0) + v
    return out


def _dmul(d, M):
    return {k: v @ M for k, v in d.items()}


def _dscale(d, s):
    return {k: s * v for k, v in d.items()}


def fold_matrices(inp):
    f64 = lambda k: np.asarray(inp[k], np.float64)
    A_T, Bw_T, By_T = f64("A_T"), f64("Bw_T"), f64("By_T")
    Cv_T, Dvw_T, Dvy_T = f64("Cv_T"), f64("Dvw_T"), f64("Dvy_T")
    Cu_T, Duw_T, Duy_T = f64("Cu_T"), f64("Duw_T"), f64("Duy_T")
    I16 = np.eye(16)
    Z16 = np.zeros((16, 16))
    X = {"XY": np.vstack([I16, Z16])}
    Y = {"XY": np.vstack([Z16, I16])}

    def K_of(Xd, s):
        return _dadd(_dmul(Xd, A_T), _dmul(Y, By_T), {f"W{s}": Bw_T})

    def C_of(Xd):
        return _dadd(_dmul(Xd, Cv_T), _dmul(Y, Dvy_T))

    K1 = K_of(X, 1)
    X2 = _dadd(X, _dscale(K1, DT / 2))
    K2 = K_of(X2, 2)
    X3 = _dadd(X, _dscale(K2, DT / 2))
    K3 = K_of(X3, 3)
    X4 = _dadd(X, _dscale(K3, DT))
    K4 = K_of(X4, 4)
    XND = _dscale(_dadd(K1, _dscale(K2, 2), _dscale(K3, 2), K4), DT / 6)
    C = [C_of(X), C_of(X2), C_of(X3), C_of(X4)]
    U = _dadd({"XY": np.vstack([Cu_T, Duy_T])}, {"W1": Duw_T})
    return {"C": C, "XND": XND, "U": U, "Dvw": Dvw_T}


def pack_blob(inp, mats):
    """Pack all bf16 stationary matrices into one [128, COLS] blob.
    Returns (blob, offsets) where offsets[name] = (k, m, col)."""
    entries = []

    def add(name, M):
        entries.append((name, np.asarray(M)))

    for s in range(4):
        for bname, M in sorted(mats["C"][s].items()):
            add(f"C{s + 1}_{bname}", M)
    for bname, M in sorted(mats["XND"].items()):
        add(f"XND_{bname}", M)
    for bname, M in sorted(mats["U"].items()):
        add(f"U_{bname}", M)
    add("Dvw", mats["Dvw"])
    add("W1mlp", np.asarray(inp["W1"], np.float64))
    add("W2mlp", np.asarray(inp["W2"], np.float64))
    add("W3mlp", np.asarray(inp["W3"], np.float64))

    col = 0
    offsets = {}
    cols_total = sum(int(M.shape[1]) for _, M in entries)
    blob = np.zeros((128, cols_total), np.float32)
    for name, M in entries:
        k, m = M.shape
        blob[:k, col:col + m] = M.astype(np.float32)
        offsets[name] = (k, m, col)
        col += m
    return blob.astype(bf16), offsets


# ---------------------------------------------------------------------------
# numpy emulator of the exact kernel dataflow (for validation / debugging)
# ---------------------------------------------------------------------------

def emulate(inp, t_steps=None):
    t_steps = t_steps or T_STEPS
    mats = fold_matrices(inp)
    r = lambda a: a.astype(bf16).astype(np.float32)  # bf16 round
    mb = {}
    for s in range(4):
        mb[f"C{s + 1}"] = {b: r(M.astype(np.float32)) for b, M in mats["C"][s].items()}
    XND = {b: r(M.astype(np.float32)) for b, M in mats["XND"].items()}
    U = {b: r(M.astype(np.float32)) for b, M in mats["U"].items()}
    Dvw = r(mats["Dvw"].astype(np.float32))

    obs = np.asarray(inp["obs"], np.float32)  # [B,T,IN]
    x = np.asarray(inp["x0"], np.float32).copy()  # [B,16] fp32 state
    means = np.zeros((B, t_steps, OUT), np.float32)
    w1_prev = None
    W = {}
    for t in range(t_steps):
        y = r(obs[:, t, :])
        XYb = np.hstack([r(x), y])  # bf16-rounded basis [B,32]
        for s in range(1, 5):
            n_it = (N1_COLD if t == 0 else N1_WARM) if s == 1 else REUSE
            base = np.float32(0)
            for b, M in mb[f"C{s}"].items():
                base = base + (XYb @ M if b == "XY" else W[b] @ M)
            w = (w1_prev if t > 0 else None) if s == 1 else W[f"W{s - 1}"]
            for i in range(n_it):
                z = base.copy()
                if w is not None:
                    z += w @ Dvw
                w = r(np.tanh(z))
            W[f"W{s}"] = w
        w1_prev = W["W1"]
        u = XYb @ U["XY"] + W["W1"] @ U["W1"]
        means[:, t, :] = u
        xnd = XYb @ XND["XY"]
        for s in range(1, 5):
            xnd = xnd + W[f"W{s}"] @ XND[f"W{s}"]
        x = x + xnd  # fp32 state update

    # value MLP (bf16 matmul inputs)
    W1m, W2m, W3m = (r(np.asarray(inp[k], np.float32)) for k in ("W1", "W2", "W3"))
    b1, b2, b3 = (np.asarray(inp[k], np.float32) for k in ("b1", "b2", "b3"))
    of = r(obs.reshape(-1, IN))
    h = r(np.tanh(of @ W1m + b1))
    h = r(np.tanh(h @ W2m + b2))
    v = (h @ W3m + b3).reshape(B, T, 1)

    ls = np.broadcast_to(np.asarray(inp["log_stds"], np.float32), means.shape)
    if t_steps == T:
        return np.concatenate([means, ls, v], -1)
    return means  # partial run: only means comparable


# ---------------------------------------------------------------------------
# Bass program
# ---------------------------------------------------------------------------

def build_program(offsets, t_steps):
    import concourse.bacc as bacc
    import concourse.mybir as mybir
    from concourse import tile

    f32 = mybir.dt.float32
    bf = mybir.dt.bfloat16
    Tanh = mybir.ActivationFunctionType.Tanh

    nc = bacc.Bacc("TRN2", target_bir_lowering=False, debug=False,
                   num_devices=NCORES)

    cols_total = max(c + m for (_, m, c) in offsets.values())
    obs_d = nc.dram_tensor("obs_t", [T, IN, BL], bf, kind="ExternalInput")
    x0_d = nc.dram_tensor("x0_t", [ST, BL], f32, kind="ExternalInput")
    wb_d = nc.dram_tensor("wblob", [128, cols_total], bf, kind="ExternalInput")
    bb_d = nc.dram_tensor("bblob", [H, 4], f32, kind="ExternalInput")
    means_d = nc.dram_tensor("means_o", [T, OUT, BL], f32, kind="ExternalOutput")
    value_d = nc.dram_tensor("value_o", [NVC, VCHUNK], f32, kind="ExternalOutput")

    with tile.TileContext(nc) as tc:
        with (
            tc.tile_pool(name="const", bufs=1) as constp,
            tc.tile_pool(name="xy", bufs=2) as xyp,
            tc.tile_pool(name="x1", bufs=2) as x1p,
            tc.tile_pool(name="wtmp", bufs=3) as wtmpp,
            tc.tile_pool(name="wfin", bufs=2) as wfinp,
            tc.tile_pool(name="sps", bufs=2, space="PSUM") as spsp,
            tc.tile_pool(name="mps", bufs=1, space="PSUM") as mpsp,
            tc.tile_pool(name="mlpps", bufs=2, space="PSUM") as mlppsp,
            tc.tile_pool(name="stage", bufs=2) as stagep,
        ):
            WB = constp.tile([128, cols_total], bf, tag="wb")
            nc.sync.dma_start(WB[:], wb_d[:])
            BB = constp.tile([H, 4], f32, tag="bb")
            nc.sync.dma_start(BB[:], bb_d[:])

            def w_ap(name):
                k, m, c = offsets[name]
                return WB[0:k, c:c + m]

            # initial state
            X1 = x1p.tile([ST, BL], f32, tag="x1")
            nc.sync.dma_start(X1[:], x0_d[:])
            XY = xyp.tile([2 * ST, BL], bf, tag="xy")
            nc.sync.dma_start(XY[ST:2 * ST, :], obs_d[0])
            nc.vector.tensor_copy(XY[0:ST, :], X1[:])

            wfin = {}       # stage name -> [128, BL] bf16 tile (this step)
            w1_prev = None  # previous step's W1 tile

            stage_wterms = {
                1: [], 2: ["W1"], 3: ["W1", "W2"], 4: ["W1", "W2", "W3"],
            }

            for t in range(t_steps):
                for s in range(1, 5):
                    n_it = (N1_COLD if t == 0 else N1_WARM) if s == 1 else REUSE
                    wf = wfinp.tile([NL, BL], bf, tag=f"wf{s}")
                    for l in range(LANES):
                        sl = slice(l * LANE_W, (l + 1) * LANE_W)
                        if s == 1:
                            wcur_ap = None if t == 0 else w1_prev[:, sl]
                        else:
                            wcur_ap = wfin[s - 1][:, sl]
                        for i in range(n_it):
                            ps = spsp.tile([NL, LANE_W], f32, tag=f"ps{l}")
                            wterms = stage_wterms[s]
                            only_mm = wcur_ap is None and not wterms
                            nc.tensor.matmul(
                                ps[:], w_ap(f"C{s}_XY"), XY[:, sl],
                                start=True, stop=only_mm)
                            for j, wb_name in enumerate(wterms):
                                lastw = wcur_ap is None and j == len(wterms) - 1
                                nc.tensor.matmul(
                                    ps[:], w_ap(f"C{s}_{wb_name}"),
                                    wfin[int(wb_name[1])][:, sl],
                                    start=False, stop=lastw)
                            if wcur_ap is not None:
                                nc.tensor.matmul(
                                    ps[:], w_ap("Dvw"), wcur_ap,
                                    start=False, stop=True)
                            if i == n_it - 1:
                                wout_ap = wf[:, sl]
                            else:
                                wtmp = wtmpp.tile(
                                    [NL, LANE_W], bf, tag=f"wt{l}", name="wtmp")
                                wout_ap = wtmp[:]
                            nc.scalar.activation(wout_ap, ps[:], Tanh)
                            wcur_ap = wout_ap
                    wfin[s] = wf

                    if s == 1:
                        # controller output u for this step
                        ups = mpsp.tile([OUT, BL], f32, tag="u")
                        nc.tensor.matmul(ups[:], w_ap("U_XY"), XY[:],
                                         start=True, stop=False)
                        nc.tensor.matmul(ups[:], w_ap("U_W1"), wf[:],
                                         start=False, stop=True)
                        ustage = stagep.tile([OUT, BL], f32, tag="us")
                        nc.vector.tensor_copy(ustage[:], ups[:])
                        nc.sync.dma_start(means_d[t], ustage[:])

                w1_prev = wfin[1]

                if t < t_steps - 1:
                    XYn = xyp.tile([2 * ST, BL], bf, tag="xy")
                    nc.sync.dma_start(XYn[ST:2 * ST, :], obs_d[t + 1])
                    xnps = mpsp.tile([ST, BL], f32, tag="xn")
                    nc.tensor.matmul(xnps[:], w_ap("XND_XY"), XY[:],
                                     start=True, stop=False)
                    for s in range(1, 5):
                        nc.tensor.matmul(xnps[:], w_ap(f"XND_W{s}"),
                                         wfin[s][:], start=False, stop=(s == 4))
                    # next-step state: bf16 into XY (critical path), fp32 copy
                    nc.vector.tensor_add(XYn[0:ST, :], X1[:], xnps[:])
                    X1n = x1p.tile([ST, BL], f32, tag="x1")
                    nc.vector.tensor_add(X1n[:], X1[:], xnps[:])
                    XY, X1 = XYn, X1n

            # ---------------- value MLP (gap-filler) ----------------
            if t_steps == T:
                for c in range(NVC):
                    mi = stagep.tile([IN, VCHUNK], bf, tag="mlpin")
                    nc.sync.dma_start(mi[:, 0:BL], obs_d[2 * c])
                    nc.sync.dma_start(mi[:, BL:2 * BL], obs_d[2 * c + 1])
                    p1 = mlppsp.tile([H, VCHUNK], f32, tag="mlpps")
                    nc.tensor.matmul(p1[:], w_ap("W1mlp"), mi[:],
                                     start=True, stop=True)
                    h1 = stagep.tile([H, VCHUNK], bf, tag="h1")
                    nc.scalar.activation(h1[:], p1[:], Tanh, bias=BB[:, 0:1])
                    p2 = mlppsp.tile([H, VCHUNK], f32, tag="mlpps")
                    nc.tensor.matmul(p2[:], w_ap("W2mlp"), h1[:],
                                     start=True, stop=True)
                    h2 = stagep.tile([H, VCHUNK], bf, tag="h2")
                    nc.scalar.activation(h2[:], p2[:], Tanh, bias=BB[:, 1:2])
                    p3 = mlppsp.tile([1, VCHUNK], f32, tag="mlpps")
                    nc.tensor.matmul(p3[:], w_ap("W3mlp"), h2[:],
                                     start=True, stop=True)
                    vs = stagep.tile([1, VCHUNK], f32, tag="vs")
                    nc.vector.tensor_scalar_add(vs[:], p3[:], float(BB_B3))
                    nc.sync.dma_start(value_d[c], vs[:])
    nc.compile()
    return nc


BB_B3 = 0.0  # replaced at build time (b3 scalar immediate)


def _prep_inputs(inputs):
    obs = np.asarray(inputs["obs"], np.float32)
    x0 = np.asarray(inputs["x0"], np.float32)
    mats = fold_matrices(inputs)
    blob, offsets = pack_blob(inputs, mats)
    bb = np.zeros((H, 4), np.float32)
    bb[:, 0] = np.asarray(inputs["b1"], np.float32)
    bb[:, 1] = np.asarray(inputs["b2"], np.float32)
    bb[0, 2] = np.asarray(inputs["b3"], np.float32)[0]

    in_maps = []
    for m in range(NCORES):
        osh = obs[m * BL:(m + 1) * BL]           # [BL, T, IN]
        obs_t = np.ascontiguousarray(
            osh.transpose(1, 2, 0)).astype(bf16)  # [T, IN, BL]
        x0_t = np.ascontiguousarray(x0[m * BL:(m + 1) * BL].T)  # [ST, BL]
        in_maps.append({
            "obs_t": obs_t, "x0_t": x0_t,
            "wblob": blob, "bblob": bb,
        })
    return in_maps, offsets


def run(inputs, t_steps=None, trace=False):
    global BB_B3
    from concourse.bass_utils import run_bass_kernel_spmd

    t_steps = t_steps or T_STEPS
    in_maps, offsets = _prep_inputs(inputs)
    BB_B3 = float(np.asarray(inputs["b3"], np.float32)[0])
    nc = build_program(offsets, t_steps)
    res = run_bass_kernel_spmd(nc, in_maps, list(range(NCORES)),
                               trace=trace)
    return res


def assemble(inputs, results, t_steps=None):
    t_steps = t_steps or T_STEPS
    means = np.zeros((B, T, OUT), np.float32)
    value = np.zeros((B, T, 1), np.float32)
    for m, r in enumerate(results):
        mo = r["means_o"]  # [T, OUT, BL]
        means[m * BL:(m + 1) * BL] = mo.transpose(2, 0, 1)
        vo = r["value_o"].reshape(T, BL)  # col = t*BL + j
        value[m * BL:(m + 1) * BL, :, 0] = vo.T
    ls = np.broadcast_to(
        np.asarray(inputs["log_stds"], np.float32), means.shape)
    return np.concatenate([means, ls, value], -1)


def kernel(**inputs):
    res = run(inputs, t_steps=T)
    return assemble(inputs, res.results)


if __name__ == "__main__":
    pass


# revision 7
# speedup vs baseline: 1.1049x; 1.1049x over previous
"""Trainium2 Bass kernel for the DissipativeRINN problem.

Strategy (pure data parallel over batch, 8 cores x 256 batch each):
  - Transposed layout on-chip: activations are [feature, batch] so every
    reference matmul  z = a @ M_T  becomes  psum = M_T.T @ aT  with the
    given *_T matrices used directly as PE stationary weights (lhsT).
  - The RK4 stage states x2/x3/x4 are never materialized: every stage's
    fixed-point offset C_s, the controller output u, and the next-step
    state delta are host-folded into small matrices over the basis
    {[x;y], w1, w2, w3, w4}, so each solve iteration is just matmuls
    into PSUM followed by one ScalarE tanh back to SBUF (bf16).
  - The cold 30-iteration solve is truncated (it converges to fp32 noise
    by ~14 iters) and warm-started from the previous step's w1.
  - Batch is further split into 2 lanes of 128 columns per core, so the
    PE->ACT->PE dependency chain of one lane overlaps the other lane.
  - The value MLP is emitted last and gap-fills idle engine slots.
"""

import numpy as np
import ml_dtypes

bf16 = ml_dtypes.bfloat16

DT = 0.01
B, T, IN, ST, NL, OUT, H = 2048, 32, 16, 16, 128, 8, 64
NCORES = 8
BL = B // NCORES  # 256

# tunables
LANES = int(__import__("os").environ.get("K_LANES", "2"))
N1_COLD = int(__import__("os").environ.get("K_N1_COLD", "15"))
N1_WARM = int(__import__("os").environ.get("K_N1_WARM", "8"))
# 0: recompute C terms each iter; 1: pre-sum static C terms (excl W_{s-1});
# 2: pre-sum all C terms incl W_{s-1}
CPRESUM = int(__import__("os").environ.get("K_CPRESUM", "0"))
REUSE = 5
T_STEPS = int(__import__("os").environ.get("K_T", str(T)))
LANE_W = BL // LANES
VCHUNK = 512
NVC = T * BL // VCHUNK  # 16 value chunks


# ---------------------------------------------------------------------------
# host-side math: fold the RK4 stage structure into matrices over the basis
# {XY (32 rows: x rows 0:16, y rows 16:32), W1, W2, W3, W4}
# row-vector convention: quantity = sum_b basis_b @ M[b]
# ---------------------------------------------------------------------------

def _dadd(*ds):
    out = {}
    for d in ds:
        for k, v in d.items():
            out[k] = out.get(k, 0) + v
    return out


def _dmul(d, M):
    return {k: v @ M for k, v in d.items()}


def _dscale(d, s):
    return {k: s * v for k, v in d.items()}


def fold_matrices(inp):
    f64 = lambda k: np.asarray(inp[k], np.float64)
    A_T, Bw_T, By_T = f64("A_T"), f64("Bw_T"), f64("By_T")
    Cv_T, Dvw_T, Dvy_T = f64("Cv_T"), f64("Dvw_T"), f64("Dvy_T")
    Cu_T, Duw_T, Duy_T = f64("Cu_T"), f64("Duw_T"), f64("Duy_T")
    I16 = np.eye(16)
    Z16 = np.zeros((16, 16))
    X = {"XY": np.vstack([I16, Z16])}
    Y = {"XY": np.vstack([Z16, I16])}

    def K_of(Xd, s):
        return _dadd(_dmul(Xd, A_T), _dmul(Y, By_T), {f"W{s}": Bw_T})

    def C_of(Xd):
        return _dadd(_dmul(Xd, Cv_T), _dmul(Y, Dvy_T))

    K1 = K_of(X, 1)
    X2 = _dadd(X, _dscale(K1, DT / 2))
    K2 = K_of(X2, 2)
    X3 = _dadd(X, _dscale(K2, DT / 2))
    K3 = K_of(X3, 3)
    X4 = _dadd(X, _dscale(K3, DT))
    K4 = K_of(X4, 4)
    XND = _dscale(_dadd(K1, _dscale(K2, 2), _dscale(K3, 2), K4), DT / 6)
    C = [C_of(X), C_of(X2), C_of(X3), C_of(X4)]
    U = _dadd({"XY": np.vstack([Cu_T, Duy_T])}, {"W1": Duw_T})
    return {"C": C, "XND": XND, "U": U, "Dvw": Dvw_T}


def pack_blob(inp, mats):
    """Pack all bf16 stationary matrices into one [128, COLS] blob.
    Returns (blob, offsets) where offsets[name] = (k, m, col)."""
    entries = []

    def add(name, M):
        entries.append((name, np.asarray(M)))

    for s in range(4):
        for bname, M in sorted(mats["C"][s].items()):
            add(f"C{s + 1}_{bname}", M)
    for bname, M in sorted(mats["XND"].items()):
        add(f"XND_{bname}", M)
    for bname, M in sorted(mats["U"].items()):
        add(f"U_{bname}", M)
    add("Dvw", mats["Dvw"])
    add("Ident", np.eye(NL))
    add("W1mlp", np.asarray(inp["W1"], np.float64))
    add("W2mlp", np.asarray(inp["W2"], np.float64))
    add("W3mlp", np.asarray(inp["W3"], np.float64))

    col = 0
    offsets = {}
    cols_total = sum(int(M.shape[1]) for _, M in entries)
    blob = np.zeros((128, cols_total), np.float32)
    for name, M in entries:
        k, m = M.shape
        blob[:k, col:col + m] = M.astype(np.float32)
        offsets[name] = (k, m, col)
        col += m
    return blob.astype(bf16), offsets


# ---------------------------------------------------------------------------
# numpy emulator of the exact kernel dataflow (for validation / debugging)
# ---------------------------------------------------------------------------

def emulate(inp, t_steps=None):
    t_steps = t_steps or T_STEPS
    mats = fold_matrices(inp)
    r = lambda a: a.astype(bf16).astype(np.float32)  # bf16 round
    mb = {}
    for s in range(4):
        mb[f"C{s + 1}"] = {b: r(M.astype(np.float32)) for b, M in mats["C"][s].items()}
    XND = {b: r(M.astype(np.float32)) for b, M in mats["XND"].items()}
    U = {b: r(M.astype(np.float32)) for b, M in mats["U"].items()}
    Dvw = r(mats["Dvw"].astype(np.float32))

    obs = np.asarray(inp["obs"], np.float32)  # [B,T,IN]
    x = np.asarray(inp["x0"], np.float32).copy()  # [B,16] fp32 state
    means = np.zeros((B, t_steps, OUT), np.float32)
    w1_prev = None
    W = {}
    for t in range(t_steps):
        y = r(obs[:, t, :])
        XYb = np.hstack([r(x), y])  # bf16-rounded basis [B,32]
        for s in range(1, 5):
            n_it = (N1_COLD if t == 0 else N1_WARM) if s == 1 else REUSE
            base = np.float32(0)
            for b, M in mb[f"C{s}"].items():
                base = base + (XYb @ M if b == "XY" else W[b] @ M)
            w = (w1_prev if t > 0 else None) if s == 1 else W[f"W{s - 1}"]
            for i in range(n_it):
                z = base.copy()
                if w is not None:
                    z += w @ Dvw
                w = r(np.tanh(z))
            W[f"W{s}"] = w
        w1_prev = W["W1"]
        u = XYb @ U["XY"] + W["W1"] @ U["W1"]
        means[:, t, :] = u
        xnd = XYb @ XND["XY"]
        for s in range(1, 5):
            xnd = xnd + W[f"W{s}"] @ XND[f"W{s}"]
        x = x + xnd  # fp32 state update

    # value MLP (bf16 matmul inputs)
    W1m, W2m, W3m = (r(np.asarray(inp[k], np.float32)) for k in ("W1", "W2", "W3"))
    b1, b2, b3 = (np.asarray(inp[k], np.float32) for k in ("b1", "b2", "b3"))
    of = r(obs.reshape(-1, IN))
    h = r(np.tanh(of @ W1m + b1))
    h = r(np.tanh(h @ W2m + b2))
    v = (h @ W3m + b3).reshape(B, T, 1)

    ls = np.broadcast_to(np.asarray(inp["log_stds"], np.float32), means.shape)
    if t_steps == T:
        return np.concatenate([means, ls, v], -1)
    return means  # partial run: only means comparable


# ---------------------------------------------------------------------------
# Bass program
# ---------------------------------------------------------------------------

def build_program(offsets, t_steps):
    import concourse.bacc as bacc
    import concourse.mybir as mybir
    from concourse import tile

    f32 = mybir.dt.float32
    bf = mybir.dt.bfloat16
    Tanh = mybir.ActivationFunctionType.Tanh

    nc = bacc.Bacc("TRN2", target_bir_lowering=False, debug=False,
                   num_devices=NCORES)

    cols_total = max(c + m for (_, m, c) in offsets.values())
    obs_d = nc.dram_tensor("obs_t", [T, IN, BL], bf, kind="ExternalInput")
    x0_d = nc.dram_tensor("x0_t", [ST, BL], f32, kind="ExternalInput")
    wb_d = nc.dram_tensor("wblob", [128, cols_total], bf, kind="ExternalInput")
    bb_d = nc.dram_tensor("bblob", [H, 4], f32, kind="ExternalInput")
    means_d = nc.dram_tensor("means_o", [T, OUT, BL], f32, kind="ExternalOutput")
    value_d = nc.dram_tensor("value_o", [NVC, VCHUNK], f32, kind="ExternalOutput")

    with tile.TileContext(nc) as tc:
        with (
            tc.tile_pool(name="const", bufs=1) as constp,
            tc.tile_pool(name="xy", bufs=2) as xyp,
            tc.tile_pool(name="x1", bufs=2) as x1p,
            tc.tile_pool(name="wtmp", bufs=3) as wtmpp,
            tc.tile_pool(name="wfin", bufs=2) as wfinp,
            tc.tile_pool(name="sps", bufs=2, space="PSUM") as spsp,
            tc.tile_pool(name="mps", bufs=1, space="PSUM") as mpsp,
            tc.tile_pool(name="mlpps", bufs=2, space="PSUM") as mlppsp,
            tc.tile_pool(name="stage", bufs=2) as stagep,
        ):
            WB = constp.tile([128, cols_total], bf, tag="wb")
            nc.sync.dma_start(WB[:], wb_d[:])
            BB = constp.tile([H, 4], f32, tag="bb")
            nc.sync.dma_start(BB[:], bb_d[:])

            def w_ap(name):
                k, m, c = offsets[name]
                return WB[0:k, c:c + m]

            # initial state
            X1 = x1p.tile([ST, BL], f32, tag="x1")
            nc.sync.dma_start(X1[:], x0_d[:])
            XY = xyp.tile([2 * ST, BL], bf, tag="xy")
            nc.sync.dma_start(XY[ST:2 * ST, :], obs_d[0])
            nc.vector.tensor_copy(XY[0:ST, :], X1[:])

            wfin = {}       # stage name -> [128, BL] bf16 tile (this step)
            w1_prev = None  # previous step's W1 tile

            stage_wterms = {
                1: [], 2: ["W1"], 3: ["W1", "W2"], 4: ["W1", "W2", "W3"],
            }

            for t in range(t_steps):
                for s in range(1, 5):
                    n_it = (N1_COLD if t == 0 else N1_WARM) if s == 1 else REUSE
                    wf = wfinp.tile([NL, BL], bf, tag=f"wf{s}")
                    for l in range(LANES):
                        sl = slice(l * LANE_W, (l + 1) * LANE_W)
                        if s == 1:
                            wcur_ap = None if t == 0 else w1_prev[:, sl]
                        else:
                            wcur_ap = wfin[s - 1][:, sl]
                        wterms = list(stage_wterms[s])
                        # optional pre-sum of C terms into a bf16 sbuf tile
                        cs_ap = None
                        presum = [] if s == 1 or CPRESUM == 0 else (
                            ["XY"] + wterms[:-1] if CPRESUM == 1
                            else ["XY"] + wterms)
                        if presum and len(presum) > 1:
                            cps = spsp.tile([NL, LANE_W], f32,
                                            tag=f"ps{l}", name="cps")
                            for j, bname in enumerate(presum):
                                src = (XY[:, sl] if bname == "XY"
                                       else wfin[int(bname[1])][:, sl])
                                nc.tensor.matmul(
                                    cps[:], w_ap(f"C{s}_{bname}"), src,
                                    start=(j == 0),
                                    stop=(j == len(presum) - 1))
                            cst = wtmpp.tile([NL, LANE_W], bf,
                                             tag=f"cs{l}", name="cst")
                            nc.vector.tensor_copy(cst[:], cps[:])
                            cs_ap = cst[:]
                            wterms = wterms[-1:] if CPRESUM == 1 else []
                        for i in range(n_it):
                            ps = spsp.tile([NL, LANE_W], f32, tag=f"ps{l}")
                            mms = []  # (lhsT name, rhs ap)
                            if cs_ap is not None:
                                mms.append(("Ident", cs_ap))
                            else:
                                mms.append((f"C{s}_XY", XY[:, sl]))
                            for wb_name in wterms:
                                mms.append((f"C{s}_{wb_name}",
                                            wfin[int(wb_name[1])][:, sl]))
                            if wcur_ap is not None:
                                mms.append(("Dvw", wcur_ap))
                            for j, (wn, rhs) in enumerate(mms):
                                nc.tensor.matmul(
                                    ps[:], w_ap(wn), rhs,
                                    start=(j == 0), stop=(j == len(mms) - 1))
                            if i == n_it - 1:
                                wout_ap = wf[:, sl]
                            else:
                                wtmp = wtmpp.tile(
                                    [NL, LANE_W], bf, tag=f"wt{l}", name="wtmp")
                                wout_ap = wtmp[:]
                            nc.scalar.activation(wout_ap, ps[:], Tanh)
                            wcur_ap = wout_ap
                    wfin[s] = wf

                    if s == 1:
                        # controller output u for this step
                        ups = mpsp.tile([OUT, BL], f32, tag="u")
                        nc.tensor.matmul(ups[:], w_ap("U_XY"), XY[:],
                                         start=True, stop=False)
                        nc.tensor.matmul(ups[:], w_ap("U_W1"), wf[:],
                                         start=False, stop=True)
                        ustage = stagep.tile([OUT, BL], f32, tag="us")
                        nc.vector.tensor_copy(ustage[:], ups[:])
                        nc.sync.dma_start(means_d[t], ustage[:])

                w1_prev = wfin[1]

                if t < t_steps - 1:
                    XYn = xyp.tile([2 * ST, BL], bf, tag="xy")
                    nc.sync.dma_start(XYn[ST:2 * ST, :], obs_d[t + 1])
                    xnps = mpsp.tile([ST, BL], f32, tag="xn")
                    nc.tensor.matmul(xnps[:], w_ap("XND_XY"), XY[:],
                                     start=True, stop=False)
                    for s in range(1, 5):
                        nc.tensor.matmul(xnps[:], w_ap(f"XND_W{s}"),
                                         wfin[s][:], start=False, stop=(s == 4))
                    # next-step state: bf16 into XY (critical path), fp32 copy
                    nc.vector.tensor_add(XYn[0:ST, :], X1[:], xnps[:])
                    X1n = x1p.tile([ST, BL], f32, tag="x1")
                    nc.vector.tensor_add(X1n[:], X1[:], xnps[:])
                    XY, X1 = XYn, X1n

            # ---------------- value MLP (gap-filler) ----------------
            if t_steps == T:
                for c in range(NVC):
                    mi = stagep.tile([IN, VCHUNK], bf, tag="mlpin")
                    nc.sync.dma_start(mi[:, 0:BL], obs_d[2 * c])
                    nc.sync.dma_start(mi[:, BL:2 * BL], obs_d[2 * c + 1])
                    p1 = mlppsp.tile([H, VCHUNK], f32, tag="mlpps")
                    nc.tensor.matmul(p1[:], w_ap("W1mlp"), mi[:],
                                     start=True, stop=True)
                    h1 = stagep.tile([H, VCHUNK], bf, tag="h1")
                    nc.scalar.activation(h1[:], p1[:], Tanh, bias=BB[:, 0:1])
                    p2 = mlppsp.tile([H, VCHUNK], f32, tag="mlpps")
                    nc.tensor.matmul(p2[:], w_ap("W2mlp"), h1[:],
                                     start=True, stop=True)
                    h2 = stagep.tile([H, VCHUNK], bf, tag="h2")
                    nc.scalar.activation(h2[:], p2[:], Tanh, bias=BB[:, 1:2])
                    p3 = mlppsp.tile([1, VCHUNK], f32, tag="mlpps")
                    nc.tensor.matmul(p3[:], w_ap("W3mlp"), h2[:],
                                     start=True, stop=True)
                    vs = stagep.tile([1, VCHUNK], f32, tag="vs")
                    nc.vector.tensor_scalar_add(vs[:], p3[:], float(BB_B3))
                    nc.sync.dma_start(value_d[c], vs[:])
    nc.compile()
    return nc


BB_B3 = 0.0  # replaced at build time (b3 scalar immediate)


def _prep_inputs(inputs):
    obs = np.asarray(inputs["obs"], np.float32)
    x0 = np.asarray(inputs["x0"], np.float32)
    mats = fold_matrices(inputs)
    blob, offsets = pack_blob(inputs, mats)
    bb = np.zeros((H, 4), np.float32)
    bb[:, 0] = np.asarray(inputs["b1"], np.float32)
    bb[:, 1] = np.asarray(inputs["b2"], np.float32)
    bb[0, 2] = np.asarray(inputs["b3"], np.float32)[0]

    in_maps = []
    for m in range(NCORES):
        osh = obs[m * BL:(m + 1) * BL]           # [BL, T, IN]
        obs_t = np.ascontiguousarray(
            osh.transpose(1, 2, 0)).astype(bf16)  # [T, IN, BL]
        x0_t = np.ascontiguousarray(x0[m * BL:(m + 1) * BL].T)  # [ST, BL]
        in_maps.append({
            "obs_t": obs_t, "x0_t": x0_t,
            "wblob": blob, "bblob": bb,
        })
    return in_maps, offsets


def run(inputs, t_steps=None, trace=False):
    global BB_B3
    from concourse.bass_utils import run_bass_kernel_spmd

    t_steps = t_steps or T_STEPS
    in_maps, offsets = _prep_inputs(inputs)
    BB_B3 = float(np.asarray(inputs["b3"], np.float32)[0])
    nc = build_program(offsets, t_steps)
    res = run_bass_kernel_spmd(nc, in_maps, list(range(NCORES)),
                               trace=trace)
    return res


def assemble(inputs, results, t_steps=None):
    t_steps = t_steps or T_STEPS
    means = np.zeros((B, T, OUT), np.float32)
    value = np.zeros((B, T, 1), np.float32)
    for m, r in enumerate(results):
        mo = r["means_o"]  # [T, OUT, BL]
        means[m * BL:(m + 1) * BL] = mo.transpose(2, 0, 1)
        vo = r["value_o"].reshape(T, BL)  # col = t*BL + j
        value[m * BL:(m + 1) * BL, :, 0] = vo.T
    ls = np.broadcast_to(
        np.asarray(inputs["log_stds"], np.float32), means.shape)
    return np.concatenate([means, ls, value], -1)


def kernel(**inputs):
    res = run(inputs, t_steps=T)
    return assemble(inputs, res.results)


if __name__ == "__main__":
    pass
